# revision 1
# baseline (speedup 1.0000x reference)
"""Trainium2 Bass kernel for CrispComposition.

Computes out[b, i] = max_o( min(m[b, i], weight[i, o]) ).

Since min(m, .) is monotone non-decreasing, the max over o commutes with it:
    max_o min(m, w[i, o]) = min(m, max_o w[i, o])
which selects one of the original values (no arithmetic), so the kernel
reduces weight over its OUT axis once (wmax[i] = max_o weight[i, o]) and
streams an elementwise min over m.

Precision: inputs are cast to bf16 host-side and the output is returned as
bf16 upcast to f32. Each output element is min(bf16(m), bf16(wmax)) — a bf16
rounding of one of the original inputs, so relative error <= 2^-9 ~= 2e-3,
well inside the 2e-2 gate. bf16 halves DMA and DVE time.

Layout: m is staged TRANSPOSED host-side (partitions = IN axis), so the
elementwise min is a per-partition tensor_scalar_min against wmax — no
on-chip broadcast of wmax along the batch axis is needed at all.

Sharding: data-parallel on the batch axis across 8 NeuronCores (batch
columns of the transposed m); weight replicated, wmax computed locally.
"""

import numpy as np
import ml_dtypes

import concourse.bacc as bacc
import concourse.mybir as mybir
from concourse.bass_utils import run_bass_kernel_spmd

from concourse.tile import TileContext

B, IN, OUT = 4096, 512, 256
NCORES = 8
BS = B // NCORES  # 512 batch columns per core
P = 128  # SBUF partitions
NT = IN // P  # 4 partition-tiles of the IN axis

BF16 = mybir.dt.bfloat16
F32 = mybir.dt.float32


def build_bass(
    m_groups=(2, 2),  # tiles per m load DMA, in tile order
    store_groups=(2, 2),  # tiles per store DMA
    load_engines=("sync", "sync", "sync", "sync"),
    store_engines=("sync", "sync", "sync", "sync"),
    reduce_mode="ttall",
    tt_stages=1,
    pool_reduce=False,
    late_ms=0,
    w_split=0,  # rows of w in a separate leading DMA (0 = single load)
):
    nc = bacc.Bacc()
    m_in = nc.declare_dram_parameter("m", [IN, BS], BF16, isOutput=False)
    w_in = nc.declare_dram_parameter("weight", [IN, OUT], BF16, isOutput=False)
    out = nc.declare_dram_parameter("out", [IN, BS], BF16, isOutput=True)

    eng = {"sync": nc.sync, "scalar": nc.scalar, "gpsimd": nc.gpsimd}

    with TileContext(nc) as tc:
        with (
            tc.tile_pool(name="consts", bufs=1) as consts,
            tc.tile_pool(name="wpool", bufs=1) as wpool,
            tc.tile_pool(name="mpool", bufs=len(m_groups)) as mpool,
            tc.tile_pool(name="opool", bufs=len(store_groups)) as opool,
        ):
            # ---- weight load + wmax[i] = max_o weight[i, o] ----
            # [IN, OUT] -> [P, NT, OUT]: partition p, slot t holds row t*P+p,
            # matching the m-tile layout below.
            # Per tile t, one fused DVE op computes
            #   scratch = max(wt[:, t, :O/2], wt[:, t, O/2:])
            #   wmax_t  = reduce_max(scratch)   (f32 accumulator)
            # tensor_tensor_reduce runs in the DVE 2-byte fast mode (the f32
            # accum is scalar-sized and exempt); wmax_t tiles are separate so
            # each min waits only its own block's reduce.
            if isinstance(w_split, tuple):
                wsplits = list(w_split)
            else:
                wsplits = []
                if w_split:
                    assert w_split % P == 0
                    wsplits.append(w_split // P)
                wsplits.append(NT - sum(wsplits))
            wmaxs = [
                consts.tile([P, 1], F32, name=f"wmax{t}", tag=f"wm{t}")
                for t in range(NT)
            ]
            wtiles = []
            tw = 0
            for k, g in enumerate(wsplits):
                wt = wpool.tile([P, g, OUT], BF16, tag=f"w{k}")
                nc.sync.dma_start(
                    out=wt,
                    in_=w_in[tw * P : (tw + g) * P, :].rearrange(
                        "(t p) o -> p t o", t=g
                    ),
                )
                wtiles.append((wt, tw, g))
                tw += g
            # tensor_tensor_reduce would fuse this but miscompiles on HW.
            # One tt-max folds the two OUT halves of ALL tiles in a single
            # DVE fast-mode op; the per-tile 128-wide reduce_max then goes
            # straight to the f32 [P, 1] scalar (scalar-sized APs are exempt
            # from the DVE 2-byte fast-mode operand check). The per-tile
            # reduces are emitted interleaved with the mins (see below) so
            # the first store group's chain is as short as possible.
            assert reduce_mode in ("ttall", "direct", "pair")
            scratch4 = consts.tile([P, NT, OUT // 2], BF16)
            scratch8 = (
                consts.tile([P, NT, OUT // 4], BF16) if tt_stages == 2 else None
            )
            red_src, red_w = (
                (scratch8, OUT // 4) if tt_stages == 2 else (scratch4, OUT // 2)
            )
            if reduce_mode in ("ttall", "pair"):
                with tc.high_priority():
                    for wt, tws, g in wtiles:
                        nc.vector.tensor_tensor(
                            out=scratch4[:, tws : tws + g, :],
                            in0=wt[:, :, : OUT // 2],
                            in1=wt[:, :, OUT // 2 :],
                            op=mybir.AluOpType.max,
                        )
                    if tt_stages == 2:
                        nc.vector.tensor_tensor(
                            out=scratch8,
                            in0=scratch4[:, :, : OUT // 4],
                            in1=scratch4[:, :, OUT // 4 :],
                            op=mybir.AluOpType.max,
                        )

            def emit_reduce(t, engine=None):
                if reduce_mode == "ttall":
                    (engine or nc.vector).reduce_max(
                        out=wmaxs[t],
                        in_=red_src[:, t, :],
                        axis=mybir.AxisListType.X,
                    )
                else:
                    src = None
                    for wt, tws, g in wtiles:
                        if tws <= t < tws + g:
                            src = wt[:, t - tws, :]
                    nc.vector.reduce_max(
                        out=wmaxs[t], in_=src, axis=mybir.AxisListType.X
                    )

            # ---- per-tile stream: load mT tile, per-partition min, store ----
            mt = {}  # tile index -> (tile, slot)
            li = 0
            t0 = 0
            for g in m_groups:
                tile = mpool.tile([P, g, BS], BF16, tag=f"m{t0}")
                eng[load_engines[li % len(load_engines)]].dma_start(
                    out=tile,
                    in_=m_in[t0 * P : (t0 + g) * P, :].rearrange(
                        "(t p) b -> p t b", t=g
                    ),
                )
                for j in range(g):
                    mt[t0 + j] = (tile, j)
                li += 1
                t0 += g

            ot = {}
            t0 = 0
            for g in store_groups:
                tile = opool.tile([P, g, BS], BF16, tag=f"o{t0}")
                for j in range(g):
                    ot[t0 + j] = (tile, j)
                t0 += g

            # Interleave reduces with mins grouped by store group: reduces
            # for group k, then mins for group k — the first group's store
            # chain doesn't wait for later groups' reduces. "pair" mode does
            # one f32-output reduce per store group (fewer DVE ops).
            t0 = 0
            for gi, g in enumerate(store_groups):
                if reduce_mode == "pair":
                    wmax_g = consts.tile(
                        [P, g], F32, name=f"wmaxg{gi}", tag=f"wmg{gi}"
                    )
                    with tc.tile_wait_until(
                        late_ms if gi > 0 else 0, enable=late_ms > 0 and gi > 0
                    ):
                        nc.vector.reduce_max(
                            out=wmax_g,
                            in_=red_src[:, t0 : t0 + g, :],
                            axis=mybir.AxisListType.X,
                        )
                    scal = lambda t: wmax_g[:, t - t0 : t - t0 + 1]
                else:
                    for t in range(t0, t0 + g):
                        eng_r = nc.gpsimd if (pool_reduce and gi > 0) else None
                        emit_reduce(t, eng_r)
                    scal = lambda t: wmaxs[t]
                for t in range(t0, t0 + g):
                    mtile, mj = mt[t]
                    otile, oj = ot[t]
                    nc.vector.tensor_scalar_min(
                        out=otile[:, oj, :],
                        in0=mtile[:, mj, :],
                        scalar1=scal(t),
                    )
                t0 += g

            si = 0
            t0 = 0
            for g in store_groups:
                tile, _ = ot[t0]
                eng[store_engines[si % len(store_engines)]].dma_start(
                    out=out[t0 * P : (t0 + g) * P, :].rearrange(
                        "(t p) b -> p t b", t=g
                    ),
                    in_=tile,
                )
                si += 1
                t0 += g

    return nc


_NC_CACHE = None


def _get_nc():
    global _NC_CACHE
    if _NC_CACHE is None:
        nc = build_bass()
        nc.finalize()
        _NC_CACHE = nc
    return _NC_CACHE


def run(m, weight, **spmd_kwargs):
    """Run the bass kernel; returns (full_output, BassKernelResults)."""
    bf16 = ml_dtypes.bfloat16
    # Host-side layout prep: transpose m so the IN axis is the partition
    # axis, and cast both inputs to bf16. All min/max compute is on-device.
    mT = np.ascontiguousarray(np.asarray(m, dtype=bf16).T)  # [IN, B]
    wb = np.ascontiguousarray(np.asarray(weight, dtype=bf16))
    nc = _get_nc()
    in_maps = [
        {"m": np.ascontiguousarray(mT[:, c * BS : (c + 1) * BS]), "weight": wb}
        for c in range(NCORES)
    ]
    res = run_bass_kernel_spmd(nc, in_maps, list(range(NCORES)), **spmd_kwargs)
    full = np.concatenate(
        [np.asarray(res.results[c]["out"]).T for c in range(NCORES)], axis=0
    )
    return full.astype(np.float32), res


def kernel(m, weight):
    return run(m, weight)[0]



# revision 5
# speedup vs baseline: 1.0514x; 1.0514x over previous
"""Trainium2 Bass kernel for CrispComposition.

Computes out[b, i] = max_o( min(m[b, i], weight[i, o]) ).

Since min(m, .) is monotone non-decreasing, the max over o commutes with it:
    max_o min(m, w[i, o]) = min(m, max_o w[i, o])
so the kernel reduces weight over its OUT axis once (wmax[i] = max_o
weight[i, o]) and streams an elementwise min over m.

Precision: inputs are cast to bf16 host-side and the output is returned as
bf16 upcast to f32. Each output element is min(bf16(m), bf16(wmax)) — a bf16
rounding of one of the original inputs (max/min select values, they don't
create new ones), so relative error <= 2^-9 ~= 2e-3, inside the 2e-2 gate.

Sharding: by the IN axis (not batch): core c owns IN rows [64c, 64c+64) for
ALL 4096 batch samples. Each core then needs only ITS 64 rows of weight
(replicated twice across the 128 partitions -> [128, 256], 64KB) instead of
the full replicated 512x256 weight (256KB) — 4x less weight traffic, and
wmax falls out of one reduce_max. m is staged host-side as [128, 2048] bf16
per core: partition p holds IN row 64c + (p % 64), batch half p // 64. The
elementwise min is a per-partition tensor_scalar_min against wmax[128, 1].

Schedule (tuned against the TRN2 instruction cost model):
  - The weight rides in ONE leading DMA together with the first m columns
    ("wm" tensor) so a single DMA-completion sem (+900ns prop) gates both
    the wmax reduce and the first min.
  - Remaining m columns load on SP (HWDGE) and optionally Pool (SWDGE,
    whose descriptor-gen runs on Pool.ENGINE in parallel with HWDGE).
  - reduce_max + per-store-chunk tensor_scalar_min on DVE.
  - Stores spread across SP/Act so HWDGE descriptor-gen (capacity 1,
    ~630ns/DMA) and the serialized DMA transfer unit (360 B/ns) stay
    saturated once the first min lands.
"""

import numpy as np
import ml_dtypes

import concourse.bacc as bacc
import concourse.mybir as mybir
from concourse.bass_utils import run_bass_kernel_spmd

from concourse.tile import TileContext

B, IN, OUT = 4096, 512, 256
NCORES = 8
RPC = IN // NCORES  # 64 IN rows per core
P = 128  # SBUF partitions
COLS = B * RPC // P  # 2048 free-dim columns per core (batch folded)

BF16 = mybir.dt.bfloat16
F32 = mybir.dt.float32

# Default schedule (see tune.py search): chosen by TimelineSim.
DEFAULT = dict(
    wm_mcols=256,
    loads=((896, "gpsimd"), (896, "sync")),
    stores=((256, "sync"), (896, "sync"), (896, "scalar")),
    wmax_dtype="bf16",
)


def build_bass(
    wm_mcols=DEFAULT["wm_mcols"],
    loads=DEFAULT["loads"],
    stores=DEFAULT["stores"],
    wmax_dtype=DEFAULT["wmax_dtype"],
):
    """wm_mcols: m columns bundled into the leading weight DMA.
    loads/stores: tuples of (ncols, engine). loads must sum to
    COLS - wm_mcols; stores must sum to COLS."""
    assert sum(c for c, _ in loads) == COLS - wm_mcols
    assert sum(c for c, _ in stores) == COLS

    nc = bacc.Bacc()
    wm_in = nc.declare_dram_parameter(
        "wm", [P, OUT + wm_mcols], BF16, isOutput=False
    )
    m_in = (
        nc.declare_dram_parameter("m", [P, COLS - wm_mcols], BF16, isOutput=False)
        if wm_mcols < COLS
        else None
    )
    out = nc.declare_dram_parameter("out", [P, COLS], BF16, isOutput=True)

    eng = {"sync": nc.sync, "scalar": nc.scalar, "gpsimd": nc.gpsimd,
           "vector": nc.vector}
    wdt = BF16 if wmax_dtype == "bf16" else F32

    with TileContext(nc) as tc:
        with (
            tc.tile_pool(name="consts", bufs=1) as consts,
            tc.tile_pool(name="wmpool", bufs=1) as wmpool,
            tc.tile_pool(name="mpool", bufs=max(1, len(loads))) as mpool,
            tc.tile_pool(name="opool", bufs=len(stores)) as opool,
        ):
            wmt = wmpool.tile([P, OUT + wm_mcols], BF16, tag="wm")
            wmax = consts.tile([P, 1], F32, name="wmax", tag="wx")
            wmaxb = (
                consts.tile([P, 1], BF16, name="wmaxb", tag="wxb")
                if wdt == BF16
                else None
            )

            # leading DMA: weight + first m columns, one completion sem
            nc.sync.dma_start(out=wmt, in_=wm_in[:, :])

            # m tiles indexed by absolute column range; the wm tile's m part
            # is the range [0, wm_mcols)
            mtiles = []
            if wm_mcols:
                mtiles.append((wmt, 0, wm_mcols, OUT))
            c0 = wm_mcols
            for ncols, e in loads:
                mt = mpool.tile([P, ncols], BF16, tag=f"m{c0}")
                eng[e].dma_start(
                    out=mt, in_=m_in[:, c0 - wm_mcols : c0 - wm_mcols + ncols]
                )
                mtiles.append((mt, c0, ncols, 0))
                c0 += ncols

            # wmax[p] = max_o w[p, o]. The bf16 variant reduces in DVE
            # 2-byte fast mode then upcasts the [P, 1] result (cheap) —
            # tensor_scalar requires an f32 scalar operand.
            if wdt == BF16:
                nc.vector.reduce_max(
                    out=wmaxb, in_=wmt[:, :OUT], axis=mybir.AxisListType.X
                )
                nc.vector.tensor_copy(out=wmax, in_=wmaxb)
            else:
                nc.vector.reduce_max(
                    out=wmax, in_=wmt[:, :OUT], axis=mybir.AxisListType.X
                )

            # per store chunk: min(s) on DVE, then store
            c0 = 0
            for ncols, e in stores:
                ot = opool.tile([P, ncols], BF16, tag=f"o{c0}")
                # emit one min per overlapped m tile (store chunks may span)
                lo = c0
                hi = c0 + ncols
                for mt, mc0, mcols, moff in mtiles:
                    a = max(lo, mc0)
                    b = min(hi, mc0 + mcols)
                    if a >= b:
                        continue
                    nc.vector.tensor_scalar_min(
                        out=ot[:, a - lo : b - lo],
                        in0=mt[:, moff + a - mc0 : moff + b - mc0],
                        scalar1=wmax,
                    )
                eng[e].dma_start(out=out[:, c0 : c0 + ncols], in_=ot)
                c0 += ncols

    return nc


_NC_CACHE = {}


def _get_nc(**kwargs):
    key = repr(sorted(kwargs.items()))
    if key not in _NC_CACHE:
        nc = build_bass(**kwargs)
        nc.finalize()
        _NC_CACHE[key] = nc
    return _NC_CACHE[key]


def shard_inputs(m, weight, wm_mcols=DEFAULT["wm_mcols"]):
    """Host-side staging: cast to bf16, transpose m so IN is the partition
    axis, fold the batch axis into partitions (2 halves), shard by IN rows.
    The first wm_mcols m columns are concatenated onto the weight tile."""
    bf16 = ml_dtypes.bfloat16
    mT = np.asarray(m, dtype=bf16).T  # [IN, B]
    wb = np.asarray(weight, dtype=bf16)  # [IN, OUT]
    half = B // 2
    in_maps = []
    for c in range(NCORES):
        rows = mT[c * RPC : (c + 1) * RPC]  # [64, B]
        m_c = np.concatenate([rows[:, :half], rows[:, half:]], axis=0)
        w_c = np.tile(wb[c * RPC : (c + 1) * RPC, :], (2, 1))  # [128, 256]
        io = {
            "wm": np.ascontiguousarray(
                np.concatenate([w_c, m_c[:, :wm_mcols]], axis=1)
            )
        }
        if wm_mcols < COLS:
            io["m"] = np.ascontiguousarray(m_c[:, wm_mcols:])
        in_maps.append(io)
    return in_maps


def unshard_output(results):
    """Per-core [128, 2048] bf16 -> full [B, IN] f32."""
    half = B // 2
    outT = np.empty((IN, B), dtype=np.float32)
    for c in range(NCORES):
        o_c = np.asarray(results[c]["out"])  # [128, 2048] bf16
        outT[c * RPC : (c + 1) * RPC, :half] = o_c[:RPC]
        outT[c * RPC : (c + 1) * RPC, half:] = o_c[RPC:]
    return np.ascontiguousarray(outT.T)


def run(m, weight, build_kwargs=None, **spmd_kwargs):
    """Run the bass kernel; returns (full_output, BassKernelResults)."""
    bk = dict(DEFAULT)
    bk.update(build_kwargs or {})
    nc = _get_nc(**bk)
    in_maps = shard_inputs(m, weight, wm_mcols=bk["wm_mcols"])
    res = run_bass_kernel_spmd(nc, in_maps, list(range(NCORES)), **spmd_kwargs)
    return unshard_output(res.results), res


def kernel(m, weight):
    return run(m, weight)[0]


# revision 19
# speedup vs baseline: 1.1385x; 1.0828x over previous
"""Trainium2 Bass kernel for CrispComposition.

Computes out[b, i] = max_o( min(m[b, i], weight[i, o]) ).

Since min(m, .) is monotone non-decreasing, the max over o commutes with it:
    max_o min(m, w[i, o]) = min(m, max_o w[i, o])
so the kernel reduces weight over its OUT axis once (wmax[i] = max_o
weight[i, o]) and streams an elementwise min over m. All min/max compute
runs on device; the host only stages layout (transpose/cast/shard).

Precision: inputs are cast to bf16 host-side and the output is returned as
bf16 upcast to f32. Each output element is min(bf16(m), bf16(wmax)) — a bf16
rounding of one of the original inputs (max/min select values, they don't
create new ones), so relative error <= 2^-9 ~= 2e-3, inside the 2e-2 gate.

Sharding: by the IN axis: core c owns IN rows [64c, 64c+64) for ALL 4096
batch samples. Each core needs only ITS 64 rows of weight (replicated twice
across the 128 partitions -> [128, 256], 64KB) instead of the full
replicated weight (256KB), and wmax falls out of one reduce_max. m is
staged host-side as [128, 2048] bf16 per core: partition p holds IN row
64c + (p % 64), batch half p // 64. The elementwise min is a per-partition
tensor_scalar_min against wmax[128, 1].

Schedule (tuned against the TRN2 instruction cost model):
  - The weight rides in ONE leading SP DMA together with the first m
    columns ("wm") so a single DMA-completion sem (+900ns prop) gates both
    the wmax reduce and the first min chunk.
  - Remaining m columns load as plain chunked DMAs (SP / Act HWDGE or Pool
    SWDGE), sized so the chunk needed LAST is small.
  - Stores use prepared SWDGE scatter writes: dma_scatter_add descriptors
    are generated EARLY (prepare_only on a dedicated SWDGE queue per
    chunk, identity iota indices, output pre-zeroed by the runtime so
    += is a plain write), and each chunk's trigger_dma fires right after
    its tensor_scalar_min lands — replacing the ~1.4us HWDGE store-issue
    path (SEQ+descgen+DGE delay) with a ~60ns Pool trigger.
"""

import numpy as np
import ml_dtypes

import concourse.bacc as bacc
import concourse.mybir as mybir
from concourse.bass_utils import run_bass_kernel_spmd

from concourse.tile import TileContext, add_dep_helper

B, IN, OUT = 4096, 512, 256
NCORES = 8
RPC = IN // NCORES  # 64 IN rows per core
P = 128  # SBUF partitions
COLS = B * RPC // P  # 2048 free-dim columns per core (batch folded)
OUT_ROWS_PAD = 256  # out DRAM rows padded: stray iota idx values (<=239)
# must stay below the row count for the scatter bounds assert

BF16 = mybir.dt.bfloat16
F32 = mybir.dt.float32
I16 = mybir.dt.int16

DEFAULT = dict(
    wm_mcols=1024,
    loads=((1024, "sync"),),
    stores=(1024, 1024),
)


def build_bass(
    wm_mcols=DEFAULT["wm_mcols"],
    loads=DEFAULT["loads"],
    stores=DEFAULT["stores"],
):
    """wm_mcols: m columns bundled into the leading weight DMA.
    loads: (ncols, engine) for the remaining m columns.
    stores: store-chunk column counts (scatter-write chunks, <= 4)."""
    assert sum(c for c, _ in loads) == COLS - wm_mcols
    assert sum(stores) == COLS
    assert len(stores) <= 4

    nc = bacc.Bacc()
    wm_in = nc.declare_dram_parameter("wm", [P, OUT + wm_mcols], BF16, isOutput=False)
    m_in = (
        nc.declare_dram_parameter("m", [P, COLS - wm_mcols], BF16, isOutput=False)
        if wm_mcols < COLS
        else None
    )
    out = nc.declare_dram_parameter("out", [OUT_ROWS_PAD, COLS], BF16, isOutput=True)

    eng = {"sync": nc.sync, "scalar": nc.scalar, "gpsimd": nc.gpsimd}

    with TileContext(nc) as tc:
        with (
            tc.tile_pool(name="consts", bufs=1) as consts,
            tc.tile_pool(name="wmpool", bufs=1) as wmpool,
            tc.tile_pool(name="mpool", bufs=max(1, len(loads))) as mpool,
            tc.tile_pool(name="opool", bufs=len(stores)) as opool,
        ):
            idx = consts.tile([P, 8], I16, tag="idx")
            wmt = wmpool.tile([P, OUT + wm_mcols], BF16, tag="wm")
            wmax = consts.tile([P, 1], F32, name="wmax", tag="wx")

            # identity scatter indices: idx[p, g] = p + 16g -> unwrapped[k]=k
            nc.gpsimd.iota(idx, pattern=[[16, 8]], base=0, channel_multiplier=1)

            # leading DMA: weight + first m columns, one completion sem
            nc.sync.dma_start(out=wmt, in_=wm_in[:, :])

            # m tiles indexed by absolute column range; the wm tile's m part
            # is the range [0, wm_mcols) at offset OUT
            mtiles = []
            if wm_mcols:
                mtiles.append((wmt, 0, wm_mcols, OUT))
            c0 = wm_mcols
            for ncols, e in loads:
                mt = mpool.tile([P, ncols], BF16, tag=f"m{c0}")
                eng[e].dma_start(
                    out=mt, in_=m_in[:, c0 - wm_mcols : c0 - wm_mcols + ncols]
                )
                mtiles.append((mt, c0, ncols, 0))
                c0 += ncols

            # output tiles + EARLY scatter-store descriptor preps (one SWDGE
            # queue per chunk; the trigger below carries the data dep)
            otiles = []
            preps = []
            prep_sem = nc.alloc_semaphore("sc_prep")
            c0 = 0
            for k, ncols in enumerate(stores):
                ot = opool.tile([P, 1, ncols], BF16, tag=f"o{c0}")
                otiles.append((ot, c0, ncols))
                sem = nc.alloc_semaphore(f"sc_dma{k}")
                prep = nc.gpsimd.dma_scatter_add(
                    out[:, c0 : c0 + ncols],
                    ot[:, :, :],
                    idx[:, :],
                    P,
                    P,
                    ncols,
                    elem_step=COLS,
                    prepare_only=True,
                    sem=sem,
                )
                preps.append(prep)
                c0 += ncols

            # wmax[p] = max_o w[p, o]
            nc.vector.reduce_max(
                out=wmax, in_=wmt[:, :OUT], axis=mybir.AxisListType.X
            )

            # per store chunk: min(s) on DVE. One trigger then fires every
            # prepared store (ring FIFO); it inherits all preps' deferred
            # source deps (Tile-managed path), so it waits the last min and
            # the store transfers pack back-to-back right after it.
            for k, (ot, c0, ncols) in enumerate(otiles):
                lo, hi = c0, c0 + ncols
                for mt, mc0, mcols, moff in mtiles:
                    a, b = max(lo, mc0), min(hi, mc0 + mcols)
                    if a >= b:
                        continue
                    nc.vector.tensor_scalar_min(
                        out=ot[:, 0, a - lo : b - lo],
                        in0=mt[:, moff + a - mc0 : moff + b - mc0],
                        scalar1=wmax,
                    )
            nc.gpsimd.trigger_dma(count=None)

    return nc


def patch_orphan_dmasw_waits(nc):
    """gen_mode==1 SWDGE preps defer their DMA-completion sem (on_update[0],
    baked into the descriptor at prep time) to trigger time, but Tile's wait
    pass still emits consumer waits against the prep's round-robin DMASW
    lane sem, which nothing increments. Rewrite those orphaned waits to the
    prep's real completion sem (same >=16 semantics)."""
    fn = nc.m.functions[0]
    insts = [i for b in fn.blocks for i in b.instructions]
    incs = {}
    for inst in insts:
        si = inst.sync_info
        if not si:
            continue
        for u in si.on_update:
            incs[u.id] = incs.get(u.id, 0) + (u.update_value or 0)
    lane_sem = {}
    k = 0
    for inst in insts:
        tn = type(inst).__name__
        if inst.engine == mybir.EngineType.Pool and (
            "DMACopy" in tn or "Gather" in tn or "Scatter" in tn
            or "Writeback" in tn or "RemoteDMA" in tn
        ):
            lane = k % 8
            k += 1
            if getattr(inst, "gen_mode", 0) == 1:
                si = inst.sync_info
                assert si and len(si.on_update) >= 1
                u0 = si.on_update[0]
                assert lane not in lane_sem, "one prep per DMASW lane"
                lane_sem[lane] = (u0.id, u0.ant_name)
    n = 0
    for inst in insts:
        si = inst.sync_info
        if not si:
            continue
        for w in si.on_wait:
            if (
                w.ant_name
                and w.ant_name.startswith("DMASW")
                and incs.get(w.id, 0) < (w.wait_value or 0)
            ):
                lane = int(w.ant_name[5:].split("_")[0])
                nid, nname = lane_sem[lane]
                w.id = nid
                w.ant_name = nname
                n += 1
    return n


_NC_CACHE = {}


def _get_nc(**kwargs):
    key = repr(sorted(kwargs.items()))
    if key not in _NC_CACHE:
        nc = build_bass(**kwargs)
        nc.finalize()
        patch_orphan_dmasw_waits(nc)
        _NC_CACHE[key] = nc
    return _NC_CACHE[key]


def shard_inputs(m, weight, wm_mcols=DEFAULT["wm_mcols"]):
    """Host-side staging: cast to bf16, transpose m so IN is the partition
    axis, fold the batch axis into partitions (2 halves), shard by IN rows.
    The first wm_mcols m columns are concatenated onto the weight tile."""
    bf16 = ml_dtypes.bfloat16
    mT = np.asarray(m, dtype=bf16).T  # [IN, B]
    wb = np.asarray(weight, dtype=bf16)  # [IN, OUT]
    half = B // 2
    in_maps = []
    for c in range(NCORES):
        rows = mT[c * RPC : (c + 1) * RPC]  # [64, B]
        m_c = np.concatenate([rows[:, :half], rows[:, half:]], axis=0)
        w_c = np.tile(wb[c * RPC : (c + 1) * RPC, :], (2, 1))  # [128, 256]
        io = {
            "wm": np.ascontiguousarray(
                np.concatenate([w_c, m_c[:, :wm_mcols]], axis=1)
            )
        }
        if wm_mcols < COLS:
            io["m"] = np.ascontiguousarray(m_c[:, wm_mcols:])
        in_maps.append(io)
    return in_maps


def unshard_output(results):
    """Per-core [OUT_ROWS_PAD, 2048] bf16 (first 128 rows valid) ->
    full [B, IN] f32."""
    half = B // 2
    outT = np.empty((IN, B), dtype=np.float32)
    for c in range(NCORES):
        o_c = np.asarray(results[c]["out"])[:P]  # [128, 2048] bf16
        outT[c * RPC : (c + 1) * RPC, :half] = o_c[:RPC]
        outT[c * RPC : (c + 1) * RPC, half:] = o_c[RPC:]
    return np.ascontiguousarray(outT.T)


def run(m, weight, build_kwargs=None, **spmd_kwargs):
    """Run the bass kernel; returns (full_output, BassKernelResults)."""
    bk = dict(DEFAULT)
    bk.update(build_kwargs or {})
    nc = _get_nc(**bk)
    in_maps = shard_inputs(m, weight, wm_mcols=bk["wm_mcols"])
    res = run_bass_kernel_spmd(nc, in_maps, list(range(NCORES)), **spmd_kwargs)
    return unshard_output(res.results), res


def kernel(m, weight):
    return run(m, weight)[0]


# revision 27
# speedup vs baseline: 1.1737x; 1.0310x over previous
"""Trainium2 Bass kernel for CrispComposition.

Computes out[b, i] = max_o( min(m[b, i], weight[i, o]) ).

Since min(m, .) is monotone non-decreasing, the max over o commutes with it:
    max_o min(m, w[i, o]) = min(m, max_o w[i, o])
so the kernel reduces weight over its OUT axis once (wmax[i] = max_o
weight[i, o]) and streams an elementwise min over m. All min/max compute
runs on device; the host only stages layout (transpose/cast/shard).

Precision: inputs are cast to bf16 host-side and the output is returned as
bf16 upcast to f32. Each output element is min(bf16(m), bf16(wmax)) — a bf16
rounding of one of the original inputs (max/min select values, they don't
create new ones), so relative error <= 2^-9 ~= 2e-3, inside the 2e-2 gate.

Sharding: by the IN axis: core c owns IN rows [64c, 64c+64) for ALL 4096
batch samples. Each core needs only ITS 64 rows of weight (replicated twice
across the 128 partitions -> [128, 256], 64KB) instead of the full
replicated weight (256KB), and wmax falls out of one reduce_max. m is
staged host-side as [128, 2048] bf16 per core: partition p holds IN row
64c + (p % 64), batch half p // 64. The elementwise min is a per-partition
tensor_scalar_min against wmax[128, 1].

Schedule (tuned against the TRN2 instruction cost model):
  - The weight rides in ONE leading SP DMA together with the first m
    columns ("wm") so a single DMA-completion sem (+900ns prop) gates both
    the wmax reduce and the first min chunk.
  - Remaining m columns load as plain chunked DMAs (SP / Act HWDGE or Pool
    SWDGE), sized so the chunk needed LAST is small.
  - Stores use prepared SWDGE scatter writes: dma_scatter_add descriptors
    are generated EARLY (prepare_only on a dedicated SWDGE queue per
    chunk, identity iota indices, output pre-zeroed by the runtime so
    += is a plain write), and each chunk's trigger_dma fires right after
    its tensor_scalar_min lands — replacing the ~1.4us HWDGE store-issue
    path (SEQ+descgen+DGE delay) with a ~60ns Pool trigger.
"""

import numpy as np
import ml_dtypes

import concourse.bacc as bacc
import concourse.mybir as mybir
from concourse.bass_utils import run_bass_kernel_spmd

from concourse.tile import TileContext, add_dep_helper

B, IN, OUT = 4096, 512, 256
NCORES = 8
RPC = IN // NCORES  # 64 IN rows per core
P = 128  # SBUF partitions
COLS = B * RPC // P  # 2048 free-dim columns per core (batch folded)
OUT_ROWS_PAD = 256  # out DRAM rows padded: stray iota idx values (<=239)
# must stay below the row count for the scatter bounds assert

BF16 = mybir.dt.bfloat16
F32 = mybir.dt.float32
I16 = mybir.dt.int16

DEFAULT = dict(
    wm_mcols=640,
    loads=((896, "gpsimd"), (512, "sync")),
    stores=(1536, 512),
    trigger_mode="paired",
)


def build_bass(
    wm_mcols=DEFAULT["wm_mcols"],
    loads=DEFAULT["loads"],
    stores=DEFAULT["stores"],
    trigger_mode=DEFAULT.get("trigger_mode", "single"),
):
    """wm_mcols: m columns bundled into the leading weight DMA.
    loads: (ncols, engine) for the remaining m columns.
    stores: store-chunk column counts (scatter-write chunks, <= 4).
    trigger_mode: "single" = all preps early, one trigger after the last
    min; "paired" = per chunk [prep_k, mins_k, trigger_k(count=None)] so
    chunk k's store fires as soon as its own min lands (official
    Tile-managed path — each trigger's pending list holds only its prep)."""
    assert sum(c for c, _ in loads) == COLS - wm_mcols
    assert sum(stores) == COLS
    assert len(stores) <= 4

    nc = bacc.Bacc()
    wm_in = nc.declare_dram_parameter("wm", [P, OUT + wm_mcols], BF16, isOutput=False)
    m_in = (
        nc.declare_dram_parameter("m", [P, COLS - wm_mcols], BF16, isOutput=False)
        if wm_mcols < COLS
        else None
    )
    out = nc.declare_dram_parameter("out", [OUT_ROWS_PAD, COLS], BF16, isOutput=True)

    eng = {"sync": nc.sync, "scalar": nc.scalar, "gpsimd": nc.gpsimd}

    with TileContext(nc) as tc:
        with (
            tc.tile_pool(name="consts", bufs=1) as consts,
            tc.tile_pool(name="wmpool", bufs=1) as wmpool,
            tc.tile_pool(name="mpool", bufs=max(1, len(loads))) as mpool,
            tc.tile_pool(name="opool", bufs=len(stores)) as opool,
        ):
            idx = consts.tile([P, 8], I16, tag="idx")
            wmt = wmpool.tile([P, OUT + wm_mcols], BF16, tag="wm")
            wmax = consts.tile([P, 1], F32, name="wmax", tag="wx")

            # identity scatter indices: idx[p, g] = p + 16g -> unwrapped[k]=k
            nc.gpsimd.iota(idx, pattern=[[16, 8]], base=0, channel_multiplier=1)

            # leading DMA: weight + first m columns, one completion sem
            nc.sync.dma_start(out=wmt, in_=wm_in[:, :])

            # m tiles indexed by absolute column range; the wm tile's m part
            # is the range [0, wm_mcols) at offset OUT
            mtiles = []
            if wm_mcols:
                mtiles.append((wmt, 0, wm_mcols, OUT))
            c0 = wm_mcols
            for ncols, e in loads:
                mt = mpool.tile([P, ncols], BF16, tag=f"m{c0}")
                eng[e].dma_start(
                    out=mt, in_=m_in[:, c0 - wm_mcols : c0 - wm_mcols + ncols]
                )
                mtiles.append((mt, c0, ncols, 0))
                c0 += ncols

            otiles = []
            c0 = 0
            for k, ncols in enumerate(stores):
                ot = opool.tile([P, 1, ncols], BF16, tag=f"o{c0}")
                otiles.append((ot, c0, ncols))
                c0 += ncols

            def emit_prep(k):
                ot, c0_, ncols = otiles[k]
                sem = nc.alloc_semaphore(f"sc_dma{k}")
                nc.gpsimd.dma_scatter_add(
                    out[:, c0_ : c0_ + ncols],
                    ot[:, :, :],
                    idx[:, :],
                    P,
                    P,
                    ncols,
                    elem_step=COLS,
                    prepare_only=True,
                    sem=sem,
                )

            def emit_mins(k):
                ot, c0_, ncols = otiles[k]
                lo, hi = c0_, c0_ + ncols
                last = None
                for mt, mc0, mcols, moff in mtiles:
                    a, b = max(lo, mc0), min(hi, mc0 + mcols)
                    if a >= b:
                        continue
                    last = nc.vector.tensor_scalar_min(
                        out=ot[:, 0, a - lo : b - lo],
                        in0=mt[:, moff + a - mc0 : moff + b - mc0],
                        scalar1=wmax,
                    )
                return last

            if trigger_mode == "split":
                # Two chunks: both preps early, so chunk 1's desc-gen runs
                # during the loads instead of gating trigger 1. Trigger 0
                # (count=1) fires prep 0's ring entry; its inherited deps
                # are backward-looking, and chunk 1's mins are emitted
                # AFTER it, so it waits only chunk 0's min. Trigger 1
                # (count=1, ring FIFO -> prep 1's entry) gets its data dep
                # on chunk 1's last min explicitly (not elidable: that min
                # is not covered by trigger 0's clock).
                assert len(stores) == 2
                emit_prep(0)
                emit_prep(1)
                nc.vector.reduce_max(
                    out=wmax, in_=wmt[:, :OUT], axis=mybir.AxisListType.X
                )
                emit_mins(0)
                trig0 = nc.gpsimd.trigger_dma(count=1)
                min1 = emit_mins(1)
                trig1 = nc.gpsimd.trigger_dma(count=1)
                add_dep_helper(trig1.ins, min1.ins, sync=True)
                add_dep_helper(trig1.ins, trig0.ins, sync=False)
            elif trigger_mode == "single":
                # all preps early; one trigger inherits every prep's
                # deferred source dep, waits the last min, then fires all
                # ring entries back-to-back
                for k in range(len(stores)):
                    emit_prep(k)
                nc.vector.reduce_max(
                    out=wmax, in_=wmt[:, :OUT], axis=mybir.AxisListType.X
                )
                for k in range(len(stores)):
                    emit_mins(k)
                nc.gpsimd.trigger_dma(count=None)
            else:
                # paired: trigger k fires right after chunk k's min. Each
                # trigger_dma(count=None) consumes exactly its own prep's
                # pending entry, so its inherited dep is just that chunk's
                # min. prep k+1's desc-gen runs behind trigger k on
                # Pool.SEQ (~1us), which gates trigger k+1 — keep chunk
                # counts low.
                emit_prep(0)
                nc.vector.reduce_max(
                    out=wmax, in_=wmt[:, :OUT], axis=mybir.AxisListType.X
                )
                emit_mins(0)
                nc.gpsimd.trigger_dma(count=None)
                for k in range(1, len(stores)):
                    emit_prep(k)
                    emit_mins(k)
                    nc.gpsimd.trigger_dma(count=None)

    return nc


def patch_orphan_dmasw_waits(nc):
    """gen_mode==1 SWDGE preps defer their DMA-completion sem (on_update[0],
    baked into the descriptor at prep time) to trigger time, but Tile's wait
    pass still emits consumer waits against the prep's round-robin DMASW
    lane sem, which nothing increments. Rewrite those orphaned waits to the
    prep's real completion sem (same >=16 semantics)."""
    fn = nc.m.functions[0]
    insts = [i for b in fn.blocks for i in b.instructions]
    incs = {}
    for inst in insts:
        si = inst.sync_info
        if not si:
            continue
        for u in si.on_update:
            incs[u.id] = incs.get(u.id, 0) + (u.update_value or 0)
    lane_sem = {}
    k = 0
    for inst in insts:
        tn = type(inst).__name__
        if inst.engine == mybir.EngineType.Pool and (
            "DMACopy" in tn or "Gather" in tn or "Scatter" in tn
            or "Writeback" in tn or "RemoteDMA" in tn
        ):
            lane = k % 8
            k += 1
            if getattr(inst, "gen_mode", 0) == 1:
                si = inst.sync_info
                assert si and len(si.on_update) >= 1
                u0 = si.on_update[0]
                assert lane not in lane_sem, "one prep per DMASW lane"
                lane_sem[lane] = (u0.id, u0.ant_name)
    n = 0
    for inst in insts:
        si = inst.sync_info
        if not si:
            continue
        for w in si.on_wait:
            if (
                w.ant_name
                and w.ant_name.startswith("DMASW")
                and incs.get(w.id, 0) < (w.wait_value or 0)
            ):
                lane = int(w.ant_name[5:].split("_")[0])
                nid, nname = lane_sem[lane]
                w.id = nid
                w.ant_name = nname
                n += 1
    return n


_NC_CACHE = {}


def _get_nc(**kwargs):
    key = repr(sorted(kwargs.items()))
    if key not in _NC_CACHE:
        nc = build_bass(**kwargs)
        nc.finalize()
        patch_orphan_dmasw_waits(nc)
        _NC_CACHE[key] = nc
    return _NC_CACHE[key]


def shard_inputs(m, weight, wm_mcols=DEFAULT["wm_mcols"]):
    """Host-side staging: cast to bf16, transpose m so IN is the partition
    axis, fold the batch axis into partitions (2 halves), shard by IN rows.
    The first wm_mcols m columns are concatenated onto the weight tile."""
    bf16 = ml_dtypes.bfloat16
    mT = np.asarray(m, dtype=bf16).T  # [IN, B]
    wb = np.asarray(weight, dtype=bf16)  # [IN, OUT]
    half = B // 2
    in_maps = []
    for c in range(NCORES):
        rows = mT[c * RPC : (c + 1) * RPC]  # [64, B]
        m_c = np.concatenate([rows[:, :half], rows[:, half:]], axis=0)
        w_c = np.tile(wb[c * RPC : (c + 1) * RPC, :], (2, 1))  # [128, 256]
        io = {
            "wm": np.ascontiguousarray(
                np.concatenate([w_c, m_c[:, :wm_mcols]], axis=1)
            )
        }
        if wm_mcols < COLS:
            io["m"] = np.ascontiguousarray(m_c[:, wm_mcols:])
        in_maps.append(io)
    return in_maps


def unshard_output(results):
    """Per-core [OUT_ROWS_PAD, 2048] bf16 (first 128 rows valid) ->
    full [B, IN] f32."""
    half = B // 2
    outT = np.empty((IN, B), dtype=np.float32)
    for c in range(NCORES):
        o_c = np.asarray(results[c]["out"])[:P]  # [128, 2048] bf16
        outT[c * RPC : (c + 1) * RPC, :half] = o_c[:RPC]
        outT[c * RPC : (c + 1) * RPC, half:] = o_c[RPC:]
    return np.ascontiguousarray(outT.T)


def run(m, weight, build_kwargs=None, **spmd_kwargs):
    """Run the bass kernel; returns (full_output, BassKernelResults)."""
    bk = dict(DEFAULT)
    bk.update(build_kwargs or {})
    nc = _get_nc(**bk)
    in_maps = shard_inputs(m, weight, wm_mcols=bk["wm_mcols"])
    res = run_bass_kernel_spmd(nc, in_maps, list(range(NCORES)), **spmd_kwargs)
    return unshard_output(res.results), res


def kernel(m, weight):
    return run(m, weight)[0]


# revision 32
# speedup vs baseline: 1.2355x; 1.0527x over previous
"""Trainium2 Bass kernel for CrispComposition.

Computes out[b, i] = max_o( min(m[b, i], weight[i, o]) ).

Since min(m, .) is monotone non-decreasing, the max over o commutes with it:
    max_o min(m, w[i, o]) = min(m, max_o w[i, o])
so the kernel reduces weight over its OUT axis once (wmax[i] = max_o
weight[i, o]) and streams an elementwise min over m. All min/max compute
runs on device; the host only stages layout (transpose/cast/shard).

Precision: inputs are cast to bf16 host-side and the output is returned as
bf16 upcast to f32. Each output element is min(bf16(m), bf16(wmax)) — a bf16
rounding of one of the original inputs (max/min select values, they don't
create new ones), so relative error <= 2^-9 ~= 2e-3, inside the 2e-2 gate.

Sharding: by the IN axis: core c owns IN rows [64c, 64c+64) for ALL 4096
batch samples. Each core needs only ITS 64 rows of weight (replicated twice
across the 128 partitions -> [128, 256], 64KB) instead of the full
replicated weight (256KB), and wmax falls out of one reduce_max. m is
staged host-side as [128, 2048] bf16 per core: partition p holds IN row
64c + (p % 64), batch half p // 64. The elementwise min is a per-partition
tensor_scalar_min against wmax[128, 1].

Schedule (tuned against the TRN2 instruction cost model):
  - The weight rides in ONE leading SP DMA together with the first m
    columns ("wm") so a single DMA-completion sem (+900ns prop) gates both
    the wmax reduce and the first min chunk.
  - Remaining m columns load as plain chunked DMAs (SP / Act HWDGE or Pool
    SWDGE), sized so the chunk needed LAST is small.
  - Stores use prepared SWDGE scatter writes: dma_scatter_add descriptors
    are generated EARLY (prepare_only on a dedicated SWDGE queue per
    chunk, identity iota indices, output pre-zeroed by the runtime so
    += is a plain write), and each chunk's trigger_dma fires right after
    its tensor_scalar_min lands — replacing the ~1.4us HWDGE store-issue
    path (SEQ+descgen+DGE delay) with a ~60ns Pool trigger.
"""

import numpy as np
import ml_dtypes

import concourse.bacc as bacc
import concourse.mybir as mybir
from concourse.bass_utils import run_bass_kernel_spmd

from concourse.tile import TileContext, add_dep_helper

B, IN, OUT = 4096, 512, 256
NCORES = 8
RPC = IN // NCORES  # 64 IN rows per core
P = 128  # SBUF partitions
COLS = B * RPC // P  # 2048 free-dim columns per core (batch folded)
OUT_ROWS_PAD = 256  # out DRAM rows padded: stray iota idx values (<=239)
# must stay below the row count for the scatter bounds assert

BF16 = mybir.dt.bfloat16
F32 = mybir.dt.float32
I16 = mybir.dt.int16

DEFAULT = dict(
    wm_mcols=512,
    loads=((512, "gpsimd"), (1024, "sync")),
    stores=(1024, 1024),
    trigger_mode="split",
)


def build_bass(
    wm_mcols=DEFAULT["wm_mcols"],
    loads=DEFAULT["loads"],
    stores=DEFAULT["stores"],
    trigger_mode=DEFAULT.get("trigger_mode", "single"),
):
    """wm_mcols: m columns bundled into the leading weight DMA.
    loads: (ncols, engine) for the remaining m columns.
    stores: store-chunk column counts (scatter-write chunks, <= 4).
    trigger_mode: "single" = all preps early, one trigger after the last
    min; "paired" = per chunk [prep_k, mins_k, trigger_k(count=None)] so
    chunk k's store fires as soon as its own min lands (official
    Tile-managed path — each trigger's pending list holds only its prep)."""
    assert sum(c for c, _ in loads) == COLS - wm_mcols
    assert sum(stores) == COLS
    assert len(stores) <= 4

    nc = bacc.Bacc()
    wm_in = nc.declare_dram_parameter("wm", [P, OUT + wm_mcols], BF16, isOutput=False)
    m_in = (
        nc.declare_dram_parameter("m", [P, COLS - wm_mcols], BF16, isOutput=False)
        if wm_mcols < COLS
        else None
    )
    out = nc.declare_dram_parameter("out", [OUT_ROWS_PAD, COLS], BF16, isOutput=True)

    eng = {"sync": nc.sync, "scalar": nc.scalar, "gpsimd": nc.gpsimd}

    with TileContext(nc) as tc:
        with (
            tc.tile_pool(name="consts", bufs=1) as consts,
            tc.tile_pool(name="wmpool", bufs=1) as wmpool,
            tc.tile_pool(name="mpool", bufs=max(1, len(loads))) as mpool,
            tc.tile_pool(name="opool", bufs=len(stores)) as opool,
        ):
            idx = consts.tile([P, 8], I16, tag="idx")
            wmt = wmpool.tile([P, OUT + wm_mcols], BF16, tag="wm")
            wmax = consts.tile([P, 1], F32, name="wmax", tag="wx")

            # identity scatter indices: idx[p, g] = p + 16g -> unwrapped[k]=k
            nc.gpsimd.iota(idx, pattern=[[16, 8]], base=0, channel_multiplier=1)

            # leading DMA: weight + first m columns, one completion sem
            nc.sync.dma_start(out=wmt, in_=wm_in[:, :])

            # m tiles indexed by absolute column range; the wm tile's m part
            # is the range [0, wm_mcols) at offset OUT
            mtiles = []
            if wm_mcols:
                mtiles.append((wmt, 0, wm_mcols, OUT))
            c0 = wm_mcols
            for ncols, e in loads:
                mt = mpool.tile([P, ncols], BF16, tag=f"m{c0}")
                eng[e].dma_start(
                    out=mt, in_=m_in[:, c0 - wm_mcols : c0 - wm_mcols + ncols]
                )
                mtiles.append((mt, c0, ncols, 0))
                c0 += ncols

            otiles = []
            c0 = 0
            for k, ncols in enumerate(stores):
                ot = opool.tile([P, 1, ncols], BF16, tag=f"o{c0}")
                otiles.append((ot, c0, ncols))
                c0 += ncols

            def emit_prep(k):
                ot, c0_, ncols = otiles[k]
                sem = nc.alloc_semaphore(f"sc_dma{k}")
                return nc.gpsimd.dma_scatter_add(
                    out[:, c0_ : c0_ + ncols],
                    ot[:, :, :],
                    idx[:, :],
                    P,
                    P,
                    ncols,
                    elem_step=COLS,
                    prepare_only=True,
                    sem=sem,
                )

            def emit_mins(k):
                ot, c0_, ncols = otiles[k]
                lo, hi = c0_, c0_ + ncols
                last = None
                for mt, mc0, mcols, moff in mtiles:
                    a, b = max(lo, mc0), min(hi, mc0 + mcols)
                    if a >= b:
                        continue
                    last = nc.vector.tensor_scalar_min(
                        out=ot[:, 0, a - lo : b - lo],
                        in0=mt[:, moff + a - mc0 : moff + b - mc0],
                        scalar1=wmax,
                    )
                return last

            if trigger_mode == "split":
                # Two chunks: both preps early, so chunk 1's desc-gen runs
                # during the loads instead of gating trigger 1. Trigger 0
                # (count=1) fires prep 0's ring entry; its inherited deps
                # are backward-looking, and chunk 1's mins are emitted
                # AFTER it, so it waits only chunk 0's min. Trigger 1
                # (count=1, ring FIFO -> prep 1's entry) gets its data dep
                # on chunk 1's last min explicitly (not elidable: that min
                # is not covered by trigger 0's clock).
                assert len(stores) == 2
                prep0 = emit_prep(0)
                prep1 = emit_prep(1)
                # pin the ring FIFO order — without this edge Tile may
                # reorder the (independent) preps and trigger k would fire
                # the WRONG chunk's entry
                add_dep_helper(prep1.ins, prep0.ins, sync=False)
                nc.vector.reduce_max(
                    out=wmax, in_=wmt[:, :OUT], axis=mybir.AxisListType.X
                )
                emit_mins(0)
                trig0 = nc.gpsimd.trigger_dma(count=1)
                min1 = emit_mins(1)
                trig1 = nc.gpsimd.trigger_dma(count=1)
                add_dep_helper(trig1.ins, min1.ins, sync=True)
                add_dep_helper(trig1.ins, trig0.ins, sync=False)
            elif trigger_mode == "single":
                # all preps early; one trigger inherits every prep's
                # deferred source dep, waits the last min, then fires all
                # ring entries back-to-back
                for k in range(len(stores)):
                    emit_prep(k)
                nc.vector.reduce_max(
                    out=wmax, in_=wmt[:, :OUT], axis=mybir.AxisListType.X
                )
                for k in range(len(stores)):
                    emit_mins(k)
                nc.gpsimd.trigger_dma(count=None)
            else:
                # paired: trigger k fires right after chunk k's min. Each
                # trigger_dma(count=None) consumes exactly its own prep's
                # pending entry, so its inherited dep is just that chunk's
                # min. prep k+1's desc-gen runs behind trigger k on
                # Pool.SEQ (~1us), which gates trigger k+1 — keep chunk
                # counts low.
                emit_prep(0)
                nc.vector.reduce_max(
                    out=wmax, in_=wmt[:, :OUT], axis=mybir.AxisListType.X
                )
                emit_mins(0)
                nc.gpsimd.trigger_dma(count=None)
                for k in range(1, len(stores)):
                    emit_prep(k)
                    emit_mins(k)
                    nc.gpsimd.trigger_dma(count=None)

    return nc


def patch_orphan_dmasw_waits(nc):
    """gen_mode==1 SWDGE preps defer their DMA-completion sem (on_update[0],
    baked into the descriptor at prep time) to trigger time, but Tile's wait
    pass still emits consumer waits against the prep's round-robin DMASW
    lane sem, which nothing increments. Rewrite those orphaned waits to the
    prep's real completion sem (same >=16 semantics)."""
    fn = nc.m.functions[0]
    insts = [i for b in fn.blocks for i in b.instructions]
    incs = {}
    for inst in insts:
        si = inst.sync_info
        if not si:
            continue
        for u in si.on_update:
            incs[u.id] = incs.get(u.id, 0) + (u.update_value or 0)
    lane_sem = {}
    k = 0
    for inst in insts:
        tn = type(inst).__name__
        if inst.engine == mybir.EngineType.Pool and (
            "DMACopy" in tn or "Gather" in tn or "Scatter" in tn
            or "Writeback" in tn or "RemoteDMA" in tn
        ):
            lane = k % 8
            k += 1
            if getattr(inst, "gen_mode", 0) == 1:
                si = inst.sync_info
                assert si and len(si.on_update) >= 1
                u0 = si.on_update[0]
                assert lane not in lane_sem, "one prep per DMASW lane"
                lane_sem[lane] = (u0.id, u0.ant_name)
    n = 0
    for inst in insts:
        si = inst.sync_info
        if not si:
            continue
        for w in si.on_wait:
            if (
                w.ant_name
                and w.ant_name.startswith("DMASW")
                and incs.get(w.id, 0) < (w.wait_value or 0)
            ):
                lane = int(w.ant_name[5:].split("_")[0])
                nid, nname = lane_sem[lane]
                w.id = nid
                w.ant_name = nname
                n += 1
    return n


def patch_split_war_waits(nc):
    """trigger_mode="split" emits chunk 1's mins AFTER the trigger that
    consumed prep 1's pending entry, so Tile adds a WAR wait on the mins:
    writer-of-otile1 waits the prep's deferred read = the s1 DMA completion
    sem — circular (that DMA fires only after the min). The hazard it
    guards is already covered: trigger 1 has an explicit sync dep on the
    last min, so the scatter can never read otile1 before the mins wrote
    it. Drop the circular wait (DVE instructions only; the final drains'
    completion waits on the same sems must stay)."""
    fn = nc.m.functions[0]
    n = 0
    for b in fn.blocks:
        for inst in b.instructions:
            if inst.engine != mybir.EngineType.DVE:
                continue
            si = inst.sync_info
            if not si:
                continue
            for w in si.on_wait:
                if (w.ant_name or "").startswith("sc_dma") and (
                    w.wait_value or 0
                ) > 0:
                    w.wait_value = 0
                    n += 1
    return n


_NC_CACHE = {}


def _get_nc(**kwargs):
    key = repr(sorted(kwargs.items()))
    if key not in _NC_CACHE:
        nc = build_bass(**kwargs)
        nc.finalize()
        patch_orphan_dmasw_waits(nc)
        patch_split_war_waits(nc)
        _NC_CACHE[key] = nc
    return _NC_CACHE[key]


def shard_inputs(m, weight, wm_mcols=DEFAULT["wm_mcols"]):
    """Host-side staging: cast to bf16, transpose m so IN is the partition
    axis, fold the batch axis into partitions (2 halves), shard by IN rows.
    The first wm_mcols m columns are concatenated onto the weight tile."""
    bf16 = ml_dtypes.bfloat16
    mT = np.asarray(m, dtype=bf16).T  # [IN, B]
    wb = np.asarray(weight, dtype=bf16)  # [IN, OUT]
    half = B // 2
    in_maps = []
    for c in range(NCORES):
        rows = mT[c * RPC : (c + 1) * RPC]  # [64, B]
        m_c = np.concatenate([rows[:, :half], rows[:, half:]], axis=0)
        w_c = np.tile(wb[c * RPC : (c + 1) * RPC, :], (2, 1))  # [128, 256]
        io = {
            "wm": np.ascontiguousarray(
                np.concatenate([w_c, m_c[:, :wm_mcols]], axis=1)
            )
        }
        if wm_mcols < COLS:
            io["m"] = np.ascontiguousarray(m_c[:, wm_mcols:])
        in_maps.append(io)
    return in_maps


def unshard_output(results):
    """Per-core [OUT_ROWS_PAD, 2048] bf16 (first 128 rows valid) ->
    full [B, IN] f32."""
    half = B // 2
    outT = np.empty((IN, B), dtype=np.float32)
    for c in range(NCORES):
        o_c = np.asarray(results[c]["out"])[:P]  # [128, 2048] bf16
        outT[c * RPC : (c + 1) * RPC, :half] = o_c[:RPC]
        outT[c * RPC : (c + 1) * RPC, half:] = o_c[RPC:]
    return np.ascontiguousarray(outT.T)


def run(m, weight, build_kwargs=None, **spmd_kwargs):
    """Run the bass kernel; returns (full_output, BassKernelResults)."""
    bk = dict(DEFAULT)
    bk.update(build_kwargs or {})
    nc = _get_nc(**bk)
    in_maps = shard_inputs(m, weight, wm_mcols=bk["wm_mcols"])
    res = run_bass_kernel_spmd(nc, in_maps, list(range(NCORES)), **spmd_kwargs)
    return unshard_output(res.results), res


def kernel(m, weight):
    return run(m, weight)[0]
